# revision 2
# baseline (speedup 1.0000x reference)
"""MetaUpscale (Meta-SR) Trainium2 kernel — bf16 + column-tiled PE version.

out[b,o,i,j] = sum_{c,ky,kx} xpad[b,c,(i//2)+ky,(j//2)+kx] * w[i*OW+j, (c*3+ky)*3+kx, o]

Shapes: x [4,64,96,96] f32, weight [36864, 576, 3] f32 -> out [4,3,192,192] f32.

Strategy (memory-bound: the 255MB weight tensor dominates):
- Shard over output rows: core r handles out rows [24r, 24r+24) i.e. source
  rows a in [12r, 12r+12).  Everything streams as bf16 (PSUM stays f32).
- The per-core weight shard (~18MB bf16) fits in SBUF: all 18 pair-blocks get
  dedicated buffers, DMAs queued upfront on the two HWDGE rings with x-slab
  chunks interleaved so the first weight block lands early.  The SDMA engines
  stream at near line rate; compute chases the DMA frontier.
- PE runs uniformly in 128x32 column-tiled mode (64x32 crashes this stack):
  every tap matmul is split into 4 concurrent 32-column tiles (8 source
  patches each, rhs N=96).  Each (group, tile) region is its own PSUM
  accumulation group (start=True on its first tap).  The K=64 leftover tap
  (ky=2,kx=2) is padded to K=128 with zero weights on the inactive partition
  half (weight rows 8/9 = [wa8;0] / [0;wb8]), so the slab's other-half
  garbage is zero-weighted.
- The four tiles of 4 consecutive groups share one PSUM bank [128, 4x96]:
  one bank holds 2 pairs' outputs and the masked extraction runs once per
  TWO pairs: DVE tensor_mul [128,384]->bf16 + reduce over the 8-patch runs
  -> [128,48].  (Weight columns are host-permuted to (cgrp, k, p%8).)
- Outputs (one [128,48] f32 tile per bank) ride SWDGE (gpsimd).
- LDWEIGHTS overlap (ldw-opt) is enabled via set_compiler_flags.
"""

import numpy as np
import ml_dtypes

import concourse.bacc as bacc
import concourse.mybir as mybir
import concourse.tile as tile
from concourse.bass_utils import run_bass_kernel_spmd

from concourse.compiler_utils import get_compiler_flags, set_compiler_flags
try:
    _flags = get_compiler_flags()
    _patched = [f.replace("--enable-ldw-opt=false", "--enable-ldw-opt=true")
                for f in _flags]
    if _patched != _flags:
        set_compiler_flags(_patched)
except Exception:
    pass

B, C, KS = 4, 64, 3
H = W = 96
OH = OW = 192
NCORES = 8
AROWS = 12            # source rows per core
HS, WS = AROWS + 2, W + 2
NP = 32               # source patches (columns) per group
NCOL = 384            # 4 cgrp x 12 k x 8 q weight columns per tap
NROWS = 10            # 8 paired taps + 2 half-padded leftover taps
NGRP = AROWS * 3      # 36 groups per core (a_loc x j_grp)
NPAIR = NGRP // 2
NBANK = NGRP // 4     # 9 PSUM banks, 4 groups (2 pairs) each

_DT = mybir.dt
_BF = ml_dtypes.bfloat16


def _build_nc():
    dt_mm = _DT.bfloat16
    nc = bacc.Bacc("TRN2", target_bir_lowering=False, debug=False)
    HEAD = 6
    xs_d = nc.dram_tensor("xs", [128, HS, WS, B], dt_mm, kind="ExternalInput").ap()
    wt_d = nc.dram_tensor("wt", [NPAIR, 128, NROWS, NCOL], dt_mm, kind="ExternalInput").ap()
    xsh_d = nc.dram_tensor("xsh", [128, AROWS, WS, B], dt_mm, kind="ExternalInput").ap()
    mask_d = nc.dram_tensor("mask", [128, NCOL], _DT.float32, kind="ExternalInput").ap()
    out_d = nc.dram_tensor("out", [128, NGRP * 12], _DT.float32, kind="ExternalOutput").ap()

    with tile.TileContext(nc) as tc:
        with (
            tc.tile_pool(name="xs", bufs=1) as xs_pool,
            tc.tile_pool(name="msk", bufs=1) as msk_pool,
            tc.tile_pool(name="res", bufs=1) as res_pool,
            tc.tile_pool(name="wt", bufs=1) as wt_pool,
            tc.tile_pool(name="tmp", bufs=3) as tmp_pool,
            tc.tile_pool(name="ps", bufs=6, space="PSUM") as ps_pool,
        ):
            # DMA order interleaves x-slab chunks between the first weight
            # blocks so wt0/wt1 land early while later pairs' x rows still
            # arrive ahead of their weight blocks.  mask + outputs ride
            # SWDGE (gpsimd).
            xh_t = xs_pool.tile([128, HEAD, WS, B], dt_mm, tag="xh")
            thh_t = xs_pool.tile([128, 4, WS, B], dt_mm, tag="thh")
            xt_t = xs_pool.tile([128, HS - HEAD, WS, B], dt_mm, tag="xt")
            tht_t = xs_pool.tile([128, 8, WS, B], dt_mm, tag="tht")
            msk_t = msk_pool.tile([128, NCOL], _DT.float32)
            nc.gpsimd.dma_start(msk_t[:], mask_d)

            wt_tiles = [wt_pool.tile([128, NROWS, NCOL], dt_mm, name=f"wt{gp}")
                        for gp in range(NPAIR)]

            # sync ring
            nc.sync.dma_start(xh_t[:, 0:3], xs_d[:, 0:3])
            nc.sync.dma_start(wt_tiles[0][:], wt_d[0])
            nc.sync.dma_start(xh_t[:, 3:HEAD], xs_d[:, 3:HEAD])
            nc.sync.dma_start(wt_tiles[2][:], wt_d[2])
            nc.sync.dma_start(tht_t[:], xsh_d[:, 4:12])
            for gp in range(4, NPAIR, 2):
                nc.sync.dma_start(wt_tiles[gp][:], wt_d[gp])
            # scalar ring
            nc.scalar.dma_start(thh_t[:, 0:1], xsh_d[:, 0:1])
            nc.scalar.dma_start(wt_tiles[1][:], wt_d[1])
            nc.scalar.dma_start(thh_t[:, 1:4], xsh_d[:, 1:4])
            nc.scalar.dma_start(wt_tiles[3][:], wt_d[3])
            nc.scalar.dma_start(xt_t[:], xs_d[:, HEAD:HS])
            for gp in range(5, NPAIR, 2):
                nc.scalar.dma_start(wt_tiles[gp][:], wt_d[gp])

            def xslab(h):
                return (xh_t, h) if h < HEAD else (xt_t, h - HEAD)

            def thslab(a_loc):
                return (thh_t, a_loc) if a_loc < 4 else (tht_t, a_loc - 4)

            for bank in range(NBANK):
                ps_t = ps_pool.tile([128, 512], _DT.float32)  # one full bank
                for h4 in range(4):
                    g = 4 * bank + h4
                    a_loc, jg = g // 3, g % 3
                    gp, halfp = g // 2, g % 2
                    wt_t = wt_tiles[gp]
                    col0 = h4 * 96
                    for cg in range(4):
                        base = jg * NP + 8 * cg
                        out_ap = ps_t[32 * cg : 32 * cg + 32, col0 : col0 + 96]
                        # 3x K=128: kx=0 on partitions 0-63 (plain slab),
                        # kx=1 on 64-127 (w+1-shifted copy)
                        for ky in range(3):
                            xt_, h = xslab(a_loc + ky)
                            nc.tensor.matmul(
                                out_ap, xt_[:, h, base : base + 8, :],
                                wt_t[:, 5 * halfp + ky, 96 * cg : 96 * cg + 96],
                                start=(ky == 0), stop=False,
                                tile_position=(0, 32 * cg),
                                skip_group_check=True,
                            )
                        # (ky=0,kx=2)+(ky=1,kx=2) via the h-shifted T_H slab
                        th_, ha = thslab(a_loc)
                        nc.tensor.matmul(
                            out_ap, th_[:, ha, base + 2 : base + 10, :],
                            wt_t[:, 5 * halfp + 3, 96 * cg : 96 * cg + 96],
                            start=False, stop=False,
                            tile_position=(0, 32 * cg),
                            skip_group_check=True,
                        )
                        # K=64 leftover (ky=2,kx=2) padded to K=128: the
                        # inactive partition half of the slab carries
                        # garbage that rows 8/9's zero half annihilates.
                        xt_, h = xslab(a_loc + 2)
                        off = base + (2 - halfp)
                        nc.tensor.matmul(
                            out_ap, xt_[:, h, off : off + 8, :],
                            wt_t[:, 5 * halfp + 4, 96 * cg : 96 * cg + 96],
                            start=False, stop=True,
                            tile_position=(0, 32 * cg),
                            skip_group_check=True,
                        )

                tmp_t = tmp_pool.tile([128, NCOL], dt_mm)
                nc.vector.tensor_mul(tmp_t[:], ps_t[:, 0:NCOL], msk_t[:])
                res_t = res_pool.tile([128, 48], _DT.float32, name=f"res{bank}")
                nc.vector.reduce_sum(
                    res_t[:],
                    tmp_t[:].rearrange("p (hk q) -> p hk q", q=8),
                    axis=mybir.AxisListType.X,
                )
                eng = nc.sync if bank % 2 == 0 else nc.scalar
                eng.dma_start(out_d[:, bank * 48 : bank * 48 + 48], res_t[:])
    nc.finalize()
    return nc


def _host_prep(x, weight):
    """Returns per-core in_maps for the 8 cores."""
    xpad = np.pad(x, ((0, 0), (0, 0), (1, 1), (1, 1)))
    # [c, h, w, b] so lhsT window columns are contiguous
    xt = np.ascontiguousarray(xpad.transpose(1, 2, 3, 0).astype(_BF))

    # weight [OH*OW, 576, 3] -> [a, di, jg, cgrp, q, dj, c, ky, kx, o]
    w10 = weight.reshape(H, 2, 3, 4, 8, 2, C, KS, KS, 3)
    # -> [a, jg, ky, kx, c, cgrp, di, dj, o, q]   (n = cgrp*96 + k*8 + q)
    wt = np.ascontiguousarray(w10.transpose(0, 2, 7, 8, 6, 3, 1, 5, 9, 4).astype(_BF))
    wt = wt.reshape(H, 3, 9, C, NCOL)

    mask = np.zeros((128, NCOL), dtype=np.float32)
    for m in range(128):
        mask[m, (m // B) % 8 :: 8] = 1.0

    xt_shift = np.zeros_like(xt)
    xt_shift[:, :, :-1] = xt[:, :, 1:]                  # slab shifted by w+1

    in_maps = []
    for r in range(NCORES):
        sl = slice(12 * r, 12 * r + HS)
        xs2 = np.concatenate([xt[:, sl], xt_shift[:, sl]], axis=0)
        wtr = wt[AROWS * r : AROWS * (r + 1)].reshape(NGRP, 9, C, NCOL)
        wa = wtr[0::2].reshape(NPAIR, 3, 3, C, NCOL)    # pair ky kx c n
        wb = wtr[1::2].reshape(NPAIR, 3, 3, C, NCOL)
        wtp = np.zeros((NPAIR, 128, NROWS, NCOL), _BF)
        wtp[:, 0:64, 0:3] = wa[:, :, 0].transpose(0, 2, 1, 3)
        wtp[:, 64:128, 0:3] = wa[:, :, 1].transpose(0, 2, 1, 3)
        wtp[:, 0:64, 3] = wa[:, 0, 2]       # T_H tap for even group
        wtp[:, 64:128, 3] = wa[:, 1, 2]
        wtp[:, 0:64, 4] = wa[:, 2, 2]       # row 4 = [wa8; 0]
        wtp[:, 0:64, 5:8] = wb[:, :, 0].transpose(0, 2, 1, 3)
        wtp[:, 64:128, 5:8] = wb[:, :, 1].transpose(0, 2, 1, 3)
        wtp[:, 0:64, 8] = wb[:, 0, 2]       # T_H tap for odd group
        wtp[:, 64:128, 8] = wb[:, 1, 2]
        wtp[:, 64:128, 9] = wb[:, 2, 2]     # row 9 = [0; wb8]
        xsh = np.concatenate([xt[:, 12 * r : 12 * r + AROWS],
                              xt[:, 12 * r + 1 : 12 * r + 1 + AROWS]], axis=0)
        in_maps.append({"xs": xs2, "xsh": xsh, "wt": wtp, "mask": mask})
    return in_maps


def _host_gather(results):
    """results: list of 8 dicts with 'out' [128, 432] -> full [B,3,OH,OW]."""
    res = np.stack([r["out"] for r in results])            # [r, 128, 432]
    res = res.reshape(NCORES, NP, B, AROWS, 3, 2, 2, 3)    # r p b a_loc jg di dj o
    out = res.transpose(2, 7, 0, 3, 5, 4, 1, 6)            # b o r a_loc di jg p dj
    return np.ascontiguousarray(out.reshape(B, 3, OH, OW))


_CACHED_NC = None


def _get_nc():
    global _CACHED_NC
    if _CACHED_NC is None:
        _CACHED_NC = _build_nc()
    return _CACHED_NC


def kernel(x, weight, **run_kwargs):
    x = np.asarray(x, dtype=np.float32)
    weight = np.asarray(weight, dtype=np.float32)
    in_maps = _host_prep(x, weight)
    nc = _get_nc()
    r = run_bass_kernel_spmd(nc, in_maps, core_ids=list(range(NCORES)), **run_kwargs)
    out = _host_gather(r.results)
    kernel.last_result = r
    return out


# revision 3
# speedup vs baseline: 1.0809x; 1.0809x over previous
"""MetaUpscale (Meta-SR) Trainium2 kernel — bf16 + column-tiled PE version.

out[b,o,i,j] = sum_{c,ky,kx} xpad[b,c,(i//2)+ky,(j//2)+kx] * w[i*OW+j, (c*3+ky)*3+kx, o]

Shapes: x [4,64,96,96] f32, weight [36864, 576, 3] f32 -> out [4,3,192,192] f32.

Strategy (memory-bound: the 255MB weight tensor dominates):
- Shard over output rows: core r handles out rows [24r, 24r+24) i.e. source
  rows a in [12r, 12r+12).  Everything streams as bf16 (PSUM stays f32).
- The per-core weight shard (~18MB bf16) fits in SBUF: all 18 pair-blocks get
  dedicated buffers, DMAs queued upfront on the two HWDGE rings (each ring
  sustains ~205GB/s, together ~410-425) with x-slab chunks interleaved so the
  first weight block lands early.  Compute chases the DMA frontier.
- The T_H slabs (row-shifted duplicates of x rows already in SBUF) are built
  ON-CHIP with gpsimd/DVE partition-shifted copies instead of being streamed
  from HBM — engine ports are disjoint from the DMA fabric, so this removes
  1.2MB from the HBM stream for free.
- PE runs uniformly in 128x32 column-tiled mode (64x32 crashes this stack):
  every tap matmul is split into 4 concurrent 32-column tiles (8 source
  patches each, rhs N=96).  Each (group, tile) region is its own PSUM
  accumulation group (start=True on its first tap).  The K=64 leftover tap
  (ky=2,kx=2) is padded to K=128 with zero weights on the inactive partition
  half (weight rows 8/9 = [wa8;0] / [0;wb8]), so the slab's other-half
  garbage is zero-weighted.
- The four tiles of 4 consecutive groups share one PSUM bank [128, 4x96]:
  one bank holds 2 pairs' outputs and the masked extraction runs once per
  TWO pairs: DVE tensor_mul [128,384]->bf16 + reduce over the 8-patch runs
  -> [128,48].  (Weight columns are host-permuted to (cgrp, k, p%8).)
- Outputs (one [128,48] f32 tile per bank) ride the HWDGE rings.
- LDWEIGHTS overlap (ldw-opt) is enabled via set_compiler_flags.
"""

import numpy as np
import ml_dtypes

import concourse.bacc as bacc
import concourse.mybir as mybir
import concourse.tile as tile
from concourse.bass_utils import run_bass_kernel_spmd

from concourse.compiler_utils import get_compiler_flags, set_compiler_flags
try:
    _flags = get_compiler_flags()
    _patched = [f.replace("--enable-ldw-opt=false", "--enable-ldw-opt=true")
                for f in _flags]
    if _patched != _flags:
        set_compiler_flags(_patched)
except Exception:
    pass

B, C, KS = 4, 64, 3
H = W = 96
OH = OW = 192
NCORES = 8
AROWS = 12            # source rows per core
HS, WS = AROWS + 2, W + 2
NP = 32               # source patches (columns) per group
NCOL = 384            # 4 cgrp x 12 k x 8 q weight columns per tap
NROWS = 10            # 8 paired taps + 2 half-padded leftover taps
NGRP = AROWS * 3      # 36 groups per core (a_loc x j_grp)
NPAIR = NGRP // 2
NBANK = NGRP // 4     # 9 PSUM banks, 4 groups (2 pairs) each

_DT = mybir.dt
_BF = ml_dtypes.bfloat16


def _build_nc():
    dt_mm = _DT.bfloat16
    nc = bacc.Bacc("TRN2", target_bir_lowering=False, debug=False)
    HEAD = 6
    xs_d = nc.dram_tensor("xs", [128, HS, WS, B], dt_mm, kind="ExternalInput").ap()
    wt_d = nc.dram_tensor("wt", [NPAIR, 128, NROWS, NCOL], dt_mm, kind="ExternalInput").ap()
    mask_d = nc.dram_tensor("mask", [128, NCOL], _DT.float32, kind="ExternalInput").ap()
    out_d = nc.dram_tensor("out", [128, NGRP * 12], _DT.float32, kind="ExternalOutput").ap()

    with tile.TileContext(nc) as tc:
        with (
            tc.tile_pool(name="xs", bufs=1) as xs_pool,
            tc.tile_pool(name="msk", bufs=1) as msk_pool,
            tc.tile_pool(name="res", bufs=1) as res_pool,
            tc.tile_pool(name="wt", bufs=1) as wt_pool,
            tc.tile_pool(name="tmp", bufs=3) as tmp_pool,
            tc.tile_pool(name="ps", bufs=6, space="PSUM") as ps_pool,
        ):
            # DMA order interleaves x-slab chunks between the first weight
            # blocks so wt0/wt1 land early while later pairs' x rows still
            # arrive ahead of their weight blocks.  mask + outputs ride
            # SWDGE (gpsimd).
            xh_t = xs_pool.tile([128, HEAD, WS, B], dt_mm, tag="xh")
            thh_t = xs_pool.tile([128, 4, WS, B], dt_mm, tag="thh")
            xt_t = xs_pool.tile([128, HS - HEAD, WS, B], dt_mm, tag="xt")
            tht_t = xs_pool.tile([128, 8, WS, B], dt_mm, tag="tht")
            msk_t = msk_pool.tile([128, NCOL], _DT.float32)
            nc.gpsimd.dma_start(msk_t[:], mask_d)

            wt_tiles = [wt_pool.tile([128, NROWS, NCOL], dt_mm, name=f"wt{gp}")
                        for gp in range(NPAIR)]

            # sync ring
            nc.sync.dma_start(xh_t[:, 0:5], xs_d[:, 0:5])
            nc.sync.dma_start(wt_tiles[0][:], wt_d[0])
            nc.sync.dma_start(xh_t[:, 5:HEAD], xs_d[:, 5:HEAD])
            for gp in range(2, NPAIR, 2):
                nc.sync.dma_start(wt_tiles[gp][:], wt_d[gp])
            # scalar ring
            nc.scalar.dma_start(wt_tiles[1][:], wt_d[1])
            nc.scalar.dma_start(xt_t[:], xs_d[:, HEAD:HS])
            for gp in range(3, NPAIR, 2):
                nc.scalar.dma_start(wt_tiles[gp][:], wt_d[gp])
            # T_H slabs (parts 64-127 = +1-row shift of the plain channels)
            # duplicate xs rows already on chip: build them with gpsimd
            # copies (engine ports are disjoint from the DMA fabric, so
            # this removes 1.2MB from the HBM stream for free).
            # thh upper rides DVE (fast, and its xh dep lands early) so
            # bank0's T_H tap isn't gated on the slow gpsimd copy chain
            nc.vector.tensor_copy(thh_t[64:128, :], xh_t[0:64, 1:5])
            nc.gpsimd.tensor_copy(thh_t[0:64, :], xh_t[0:64, 0:4])
            nc.gpsimd.tensor_copy(tht_t[0:64, 0:2], xh_t[0:64, 4:HEAD])
            nc.gpsimd.tensor_copy(tht_t[0:64, 2:8], xt_t[0:64, 0:6])
            nc.gpsimd.tensor_copy(tht_t[64:128, 0:1], xh_t[0:64, 5:HEAD])
            nc.gpsimd.tensor_copy(tht_t[64:128, 1:8], xt_t[0:64, 0:7])

            def xslab(h):
                return (xh_t, h) if h < HEAD else (xt_t, h - HEAD)

            def thslab(a_loc):
                return (thh_t, a_loc) if a_loc < 4 else (tht_t, a_loc - 4)

            for bank in range(NBANK):
                ps_t = ps_pool.tile([128, 512], _DT.float32)  # one full bank
                for h4 in range(4):
                    g = 4 * bank + h4
                    a_loc, jg = g // 3, g % 3
                    gp, halfp = g // 2, g % 2
                    wt_t = wt_tiles[gp]
                    col0 = h4 * 96
                    for cg in range(4):
                        base = jg * NP + 8 * cg
                        out_ap = ps_t[32 * cg : 32 * cg + 32, col0 : col0 + 96]
                        # 3x K=128: kx=0 on partitions 0-63 (plain slab),
                        # kx=1 on 64-127 (w+1-shifted copy)
                        for ky in range(3):
                            xt_, h = xslab(a_loc + ky)
                            nc.tensor.matmul(
                                out_ap, xt_[:, h, base : base + 8, :],
                                wt_t[:, 5 * halfp + ky, 96 * cg : 96 * cg + 96],
                                start=(ky == 0), stop=False,
                                tile_position=(0, 32 * cg),
                                skip_group_check=True,
                            )
                        # (ky=0,kx=2)+(ky=1,kx=2) via the h-shifted T_H slab
                        th_, ha = thslab(a_loc)
                        nc.tensor.matmul(
                            out_ap, th_[:, ha, base + 2 : base + 10, :],
                            wt_t[:, 5 * halfp + 3, 96 * cg : 96 * cg + 96],
                            start=False, stop=False,
                            tile_position=(0, 32 * cg),
                            skip_group_check=True,
                        )
                        # K=64 leftover (ky=2,kx=2) padded to K=128: the
                        # inactive partition half of the slab carries
                        # garbage that rows 8/9's zero half annihilates.
                        xt_, h = xslab(a_loc + 2)
                        off = base + (2 - halfp)
                        nc.tensor.matmul(
                            out_ap, xt_[:, h, off : off + 8, :],
                            wt_t[:, 5 * halfp + 4, 96 * cg : 96 * cg + 96],
                            start=False, stop=True,
                            tile_position=(0, 32 * cg),
                            skip_group_check=True,
                        )

                tmp_t = tmp_pool.tile([128, NCOL], dt_mm)
                nc.vector.tensor_mul(tmp_t[:], ps_t[:, 0:NCOL], msk_t[:])
                res_t = res_pool.tile([128, 48], _DT.float32, name=f"res{bank}")
                nc.vector.reduce_sum(
                    res_t[:],
                    tmp_t[:].rearrange("p (hk q) -> p hk q", q=8),
                    axis=mybir.AxisListType.X,
                )
                eng = nc.sync if bank % 2 == 0 else nc.scalar
                eng.dma_start(out_d[:, bank * 48 : bank * 48 + 48], res_t[:])
    nc.finalize()
    return nc


def _host_prep(x, weight):
    """Returns per-core in_maps for the 8 cores."""
    xpad = np.pad(x, ((0, 0), (0, 0), (1, 1), (1, 1)))
    # [c, h, w, b] so lhsT window columns are contiguous
    xt = np.ascontiguousarray(xpad.transpose(1, 2, 3, 0).astype(_BF))

    # weight [OH*OW, 576, 3] -> [a, di, jg, cgrp, q, dj, c, ky, kx, o]
    w10 = weight.reshape(H, 2, 3, 4, 8, 2, C, KS, KS, 3)
    # -> [a, jg, ky, kx, c, cgrp, di, dj, o, q]   (n = cgrp*96 + k*8 + q)
    wt = np.ascontiguousarray(w10.transpose(0, 2, 7, 8, 6, 3, 1, 5, 9, 4).astype(_BF))
    wt = wt.reshape(H, 3, 9, C, NCOL)

    mask = np.zeros((128, NCOL), dtype=np.float32)
    for m in range(128):
        mask[m, (m // B) % 8 :: 8] = 1.0

    xt_shift = np.zeros_like(xt)
    xt_shift[:, :, :-1] = xt[:, :, 1:]                  # slab shifted by w+1

    in_maps = []
    for r in range(NCORES):
        sl = slice(12 * r, 12 * r + HS)
        xs2 = np.concatenate([xt[:, sl], xt_shift[:, sl]], axis=0)
        wtr = wt[AROWS * r : AROWS * (r + 1)].reshape(NGRP, 9, C, NCOL)
        wa = wtr[0::2].reshape(NPAIR, 3, 3, C, NCOL)    # pair ky kx c n
        wb = wtr[1::2].reshape(NPAIR, 3, 3, C, NCOL)
        wtp = np.zeros((NPAIR, 128, NROWS, NCOL), _BF)
        wtp[:, 0:64, 0:3] = wa[:, :, 0].transpose(0, 2, 1, 3)
        wtp[:, 64:128, 0:3] = wa[:, :, 1].transpose(0, 2, 1, 3)
        wtp[:, 0:64, 3] = wa[:, 0, 2]       # T_H tap for even group
        wtp[:, 64:128, 3] = wa[:, 1, 2]
        wtp[:, 0:64, 4] = wa[:, 2, 2]       # row 4 = [wa8; 0]
        wtp[:, 0:64, 5:8] = wb[:, :, 0].transpose(0, 2, 1, 3)
        wtp[:, 64:128, 5:8] = wb[:, :, 1].transpose(0, 2, 1, 3)
        wtp[:, 0:64, 8] = wb[:, 0, 2]       # T_H tap for odd group
        wtp[:, 64:128, 8] = wb[:, 1, 2]
        wtp[:, 64:128, 9] = wb[:, 2, 2]     # row 9 = [0; wb8]
        in_maps.append({"xs": xs2, "wt": wtp, "mask": mask})
    return in_maps


def _host_gather(results):
    """results: list of 8 dicts with 'out' [128, 432] -> full [B,3,OH,OW]."""
    res = np.stack([r["out"] for r in results])            # [r, 128, 432]
    res = res.reshape(NCORES, NP, B, AROWS, 3, 2, 2, 3)    # r p b a_loc jg di dj o
    out = res.transpose(2, 7, 0, 3, 5, 4, 1, 6)            # b o r a_loc di jg p dj
    return np.ascontiguousarray(out.reshape(B, 3, OH, OW))


_CACHED_NC = None


def _get_nc():
    global _CACHED_NC
    if _CACHED_NC is None:
        _CACHED_NC = _build_nc()
    return _CACHED_NC


def kernel(x, weight, **run_kwargs):
    x = np.asarray(x, dtype=np.float32)
    weight = np.asarray(weight, dtype=np.float32)
    in_maps = _host_prep(x, weight)
    nc = _get_nc()
    r = run_bass_kernel_spmd(nc, in_maps, core_ids=list(range(NCORES)), **run_kwargs)
    out = _host_gather(r.results)
    kernel.last_result = r
    return out
